# revision 30
# baseline (speedup 1.0000x reference)
"""Trainium2 Bass kernel for nn_ConvPair (pairwise-MLP message passing).

Reference computation (N=1024 atoms, F=8 feats, H=128 hidden, O=3 out):
    hi = x @ W1[:F];  hj = x @ W1[F:]
    h  = tanh(hi[:,None,:] + hj[None,:,:] + b1)        # [N,N,H]
    h  = tanh(h @ W2 + b2)                             # [N,N,H]
    y  = tanh(h @ W3 + b3)                             # [N,N,O]
    out = y.sum(axis=(1,2))                            # [N]

Sharding: outer atom dim i split across 8 cores (128 i per core); weights and
the (host-precomputed) j-side tables replicated. No cross-core reduction.

Design (vs the fp32 all-ACT baseline at ~790us):
  * all N^2-scale matmuls in bf16 (PE: 1 cycle/row instead of 4 for fp32;
    mm1 moving operand still capped at 512 fp32 PSUM cols per matmul),
  * layer-1 tanh per atom as ONE short ACT instruction using the free
    affine bias port: h1_i = tanh(HJ + hib_i). Short (1K-col) ACT
    instructions measured ~0.35-1.0 ns/col on this part, while LONG ones
    (8K-col batched) run at exactly (N+352)/1.2 ns = 1 col/cycle — so
    batching tanh1 into big instructions is a LOSS; per-atom instructions
    also remove any DVE dependency from the critical path,
  * layer-2 tanh reads PSUM directly with the b2 bias folded in, writes
    bf16; stage-3 uses pairs-on-partitions (h2 128-pair chunk stationary,
    W3pad moving) so tanh3 touches only 32 PSUM cols per atom,
  * per-group (8 atoms) tail: tanh3 in-place on PSUM, DVE reduce, then
    accumulate into ACC (accumulation keeps reps>1 timing variants alive
    against walrus dead-store elimination; reps=1 output is unchanged),
  * final j-offset reduction via one PE matmul (ACC.T @ ones).

Engine budget per core (measured op costs, 128 atoms): PE ~90-140us
(mm1 2x512 rows + 8 stationary swaps for mm3), ACT ~90-290us depending on
short-instruction overlap, DVE ~10us. Rejected alternatives: DVE identity
tanh via reciprocal (nc.vector.reciprocal measured 6.4us per 1K tile);
custom DVE ops (reciprocal_approx_fast) do not compile under this walrus
("ISA wrong length").

b1 is folded into hib host-side; b2 via the ACT bias port; b3 is zeros for
this problem (asserted; a numpy fallback handles the hypothetical nonzero
case exactly).

Wait-discipline: walrus's codegen supports limited semaphore waits per
instruction, so `_legalize_waits` hoists excess waits onto chained NoOps.
"""

import json

import numpy as np
import ml_dtypes
from contextlib import ExitStack

import bass_rust
import concourse.bass as bass
import concourse.tile as tile
from concourse import mybir
from concourse.bass_utils import run_bass_kernel_spmd

f32 = mybir.dt.float32
bf16 = mybir.dt.bfloat16
Tanh = mybir.ActivationFunctionType.Tanh
ADD = mybir.AluOpType.add
MULT = mybir.AluOpType.mult

N, F, H, O = 1024, 8, 128, 3
NCORES = 8
IPC = N // NCORES  # 128 atoms (i) per core
NJ = N             # full j dimension on every core
OPAD = 4           # W3 padded 3 -> 4 cols (pad col = 0)
G = 8              # cohort size (i's per A-group / D-cohort)
NBLK = IPC // (2 * G)  # 8 blocks: each covers one A-group + one D-cohort
NCH = NJ // H      # 8 pair-chunks of 128 j per stage-3 matmul
MMN = 512          # mm1 moving chunk (PSUM bank = 512 fp32)

# bf16 const block column offsets (HJ has a 128-col wrap for mmonly APs)
B_HJ, B_W2, B_W3 = 0, NJ + H, NJ + 2 * H
BCOLS = NJ + 2 * H + OPAD
# f32 const block column offsets
F_HIB, F_B2, F_ONES = 0, IPC, IPC + 1
FCOLS = IPC + 2

# TPB instructions have a single 8-byte events field: 2 sync commands max
# (walrus rejects more).  Queue-engine DMA ops handle their own sync.
_MULTIWAIT_OK = {"DMACopy", "TriggeredCopy", "Call", "ISA"}


def _legalize_waits(nc):
    """Hoist excess semaphore waits from datapath instructions onto chained
    NoOps (one wait each) so every instruction fits walrus's sync budget."""
    j = json.loads(bass_rust.module_to_json_string(nc.m))
    counter = [0]

    def fix_list(insts):
        out = []
        for inst in insts:
            si = inst.get("sync_info")
            waits = (si or {}).get("on_wait", [])
            if si and len(waits) > 1 and inst.get("opcode") not in _MULTIWAIT_OK:
                for w in waits:
                    counter[0] += 1
                    out.append({
                        "debug": inst.get("debug", 0),
                        "engine": inst["engine"],
                        "ins": [],
                        "outs": [],
                        "name": f"W-hoist-{counter[0]}",
                        "opcode": "NoOp",
                        "sync_info": {"on_update": [], "on_wait": [w]},
                    })
                si["on_wait"] = []
            out.append(inst)
        return out

    def walk(o):
        if isinstance(o, dict):
            if "instructions" in o and isinstance(o["instructions"], list):
                o["instructions"] = fix_list(o["instructions"])
            for v in o.values():
                walk(v)
        elif isinstance(o, list):
            for v in o:
                walk(v)

    walk(j)
    nc.m = bass_rust.module_from_json_string(json.dumps(j))
    return counter[0]


def _build(reps=1):
    """Build the per-core Bass program (SPMD: same program, per-core data).

    reps > 1 repeats the main loop (recomputing identical results); used
    only for differential timing.

    NN_CONVPAIR_TIMING_VARIANT env (timing experiments ONLY — wrong results): "batched" uses DVE
    adds + one 8K ACT tanh per group; "not1"/"not2"/"nomm3" skip stages;
    "t1only"/"t1only_batched" measure the tanh1 stream alone; "mmonly"
    measures the PE stream alone."""
    import os
    variant = os.environ.get("NN_CONVPAIR_TIMING_VARIANT", "full")
    nc = bass.Bass()
    cbparam = nc.declare_dram_parameter("cb", [H, BCOLS], bf16, isOutput=False)
    cfparam = nc.declare_dram_parameter("cf", [H, FCOLS], f32, isOutput=False)
    yparam = nc.declare_dram_parameter("y", [IPC, 1], f32, isOutput=True)

    with tile.TileContext(nc) as tc:
        with ExitStack() as ctx:
            cbp = ctx.enter_context(tc.tile_pool(name="cbp", bufs=1))
            cfp = ctx.enter_context(tc.tile_pool(name="cfp", bufs=1))
            h1p = ctx.enter_context(tc.tile_pool(name="h1p", bufs=3))
            h2p = ctx.enter_context(tc.tile_pool(name="h2p", bufs=3))
            accp = ctx.enter_context(tc.tile_pool(name="accp", bufs=1))
            scrp = ctx.enter_context(tc.tile_pool(name="scrp", bufs=1))
            # PSUM budget (8 banks): ps1 2 bufs x 2 banks + ps3 3 x 1
            # + warm/fin 1
            ps1p = ctx.enter_context(tc.tile_pool(name="ps1p", bufs=2,
                                                  space="PSUM"))
            ps3p = ctx.enter_context(tc.tile_pool(name="ps3p", bufs=3,
                                                  space="PSUM"))
            psW = ctx.enter_context(tc.tile_pool(name="psW", bufs=1,
                                                 space="PSUM"))

            CB = cbp.tile([H, BCOLS], bf16)
            nc.sync.dma_start(out=CB, in_=cbparam[:, :])
            CF = cfp.tile([H, FCOLS], f32)
            nc.sync.dma_start(out=CF, in_=cfparam[:, :])

            HJ = CB[:, B_HJ:B_HJ + NJ]
            W2 = CB[:, B_W2:B_W2 + H]
            W3 = CB[:, B_W3:B_W3 + OPAD]
            B2 = CF[:, F_B2:F_B2 + 1]
            ONES = CF[:, F_ONES:F_ONES + 1]

            ACC = accp.tile([H, IPC], f32)          # [j-offset, i] partials
            warm = scrp.tile([H, 1], f32, tag="warm")
            warmb = scrp.tile([H, 1], bf16, tag="warmb")

            # --- warmups: every engine observes both const DMAs on
            # single-wait instructions; first Tanh loads the ACT table.
            nc.scalar.activation(out=warm, in_=B2, func=Tanh)
            nc.scalar.activation(out=warmb, in_=CB[:, 0:1], func=Tanh)
            warm_ps = psW.tile([IPC, 1], f32, tag="wf")
            nc.tensor.matmul(warm_ps[0:1, 0:1], CB[:, B_W2:B_W2 + 1],
                             CB[:, B_W2:B_W2 + 1], start=True, stop=True)
            nc.tensor.matmul(warm_ps[0:1, 0:1], ONES, ONES,
                             start=True, stop=True)
            warmd = scrp.tile([H, 1], f32, tag="warmd")
            nc.vector.tensor_scalar_add(warmd, CB[:, 0:1], CF[:, 0:1])
            # ACC is accumulated into (not overwritten) so that with reps>1
            # every rep stays live (defeats walrus DCE in timing variants).
            nc.vector.memset(ACC, 0.0)
            redp = ctx.enter_context(tc.tile_pool(name="redp", bufs=2))

            def emit_group_h1(g):
                """Group h1 production. Default: per-atom ACT tanh via the
                bias port, split into 512-col halves (short ACT instructions
                overlap on HW; long ones run at 1 col/cycle).
                variant=batched: DVE adds + one big ACT tanh (slow: 8K ACT)."""
                h1g = h1p.tile([H, G, NJ], bf16)
                for k in range(G):
                    t = G * g + k
                    bias = CF[:, F_HIB + t:F_HIB + t + 1]
                    if variant in ("batched", "t1only_batched"):
                        nc.vector.tensor_scalar_add(h1g[:, k, :], HJ, bias)
                    else:
                        nc.scalar.activation(out=h1g[:, k, :], in_=HJ,
                                             func=Tanh, bias=bias)
                if variant in ("batched", "t1only_batched"):
                    nc.scalar.activation(out=h1g[:, :, :], in_=h1g[:, :, :],
                                         func=Tanh)
                return h1g

            mm1_pm = (mybir.MatmulPerfMode.DoublePixel
                      if variant == "dpix" else None)

            def emit_mm1_t2(h1):
                """PE mm1 -> ACT tanh2 for one atom; mm3 deferred so the
                PE can fill tanh2's latency with the PREVIOUS atom's mm3s
                (one-atom software pipeline)."""
                ps1 = ps1p.tile([H, NJ], f32)
                h2 = h2p.tile([H, NJ], bf16)
                for m in range(NJ // MMN):
                    sl = slice(m * MMN, (m + 1) * MMN)
                    nc.tensor.matmul(ps1[:, sl], W2, h1[:, sl],
                                     start=True, stop=True, perf_mode=mm1_pm)
                if variant != "not2":
                    nc.scalar.activation(out=h2, in_=ps1, func=Tanh, bias=B2)
                return h2

            def emit_mm3(h2, ps3, slot):
                for c in range(NCH):
                    nc.tensor.matmul(ps3[:, slot, c, :],
                                     h2[:, c * H:(c + 1) * H], W3,
                                     start=True, stop=True)

            def emit_mm(h1, ps3, slot):
                emit_mm3(emit_mm1_t2(h1), ps3, slot)

            def emit_tail(ps3, col0):
                """ACT tanh3 in-place on PSUM; DVE reduce + accumulate."""
                nc.scalar.activation(out=ps3[:, :, :, :], in_=ps3[:, :, :, :],
                                     func=Tanh)
                red = redp.tile([H, G], f32)
                nc.vector.tensor_reduce(
                    out=red,
                    in_=ps3.rearrange("p g c o -> p g (c o)"),
                    axis=mybir.AxisListType.X, op=ADD)
                nc.vector.tensor_tensor(out=ACC[:, col0:col0 + G],
                                        in0=ACC[:, col0:col0 + G],
                                        in1=red, op=ADD)

            for _ in range(reps):
                if variant == "null":
                    break
                if variant in ("t1only", "t1only_batched"):
                    # measure the tanh1 stream alone; accumulating reduce
                    # keeps every h1g live (defeats DCE).
                    for g in range(IPC // G):
                        h1g = emit_group_h1(g)
                        red = redp.tile([H, G], f32)
                        # touch every k-slice so no tanh1 is dead code
                        nc.vector.tensor_reduce(
                            out=red, in_=h1g[:, :, 0:2],
                            axis=mybir.AxisListType.X, op=ADD)
                        nc.vector.tensor_tensor(out=ACC[:, g * G:(g + 1) * G],
                                                in0=ACC[:, g * G:(g + 1) * G],
                                                in1=red, op=ADD)
                    continue
                if variant == "mmonly":
                    # measure the PE stream alone; unique AP offsets per i
                    # (HJ wrap region) defeat elision; no tanh1/tanh2.
                    for g in range(IPC // G):
                        ps3 = ps3p.tile([H, G, NCH, OPAD], f32, tag="s3")
                        for k in range(G):
                            t = G * g + k
                            off = t % H
                            ps1 = ps1p.tile([H, NJ], f32)
                            for m in range(NJ // MMN):
                                nc.tensor.matmul(
                                    ps1[:, m * MMN:(m + 1) * MMN], W2,
                                    CB[:, off + m * MMN:off + (m + 1) * MMN],
                                    start=True, stop=True)
                            for c in range(NCH):
                                nc.tensor.matmul(
                                    ps3[:, k, c, :],
                                    CB[:, off + c * H:off + (c + 1) * H], W3,
                                    start=True, stop=True)
                        emit_tail(ps3, G * g)
                    continue
                # software pipeline: group g+1's h1 (DVE adds + one batched
                # ACT tanh) is produced while group g runs mm1/tanh2/mm3.
                h1g_next = None if variant == "not1" else emit_group_h1(0)
                pend = None          # (h2, ps3, slot) awaiting its mm3s
                tailq = []           # ps3 tiles whose last mm3 was emitted
                for g in range(IPC // G):
                    h1g = h1g_next
                    if g + 1 < IPC // G and variant != "not1":
                        h1g_next = emit_group_h1(g + 1)
                    ps3 = ps3p.tile([H, G, NCH, OPAD], f32, tag="s3")
                    for k in range(G):
                        h1 = HJ if variant == "not1" else h1g[:, k, :]
                        if variant == "nomm3":
                            emit_mm1_t2(h1)
                            continue
                        h2 = emit_mm1_t2(h1)
                        if pend is not None:
                            emit_mm3(pend[0], pend[1], pend[2])
                            if pend[2] == G - 1:     # finished a group
                                tailq.append((pend[1], pend[3]))
                        pend = (h2, ps3, k, G * g)
                        while tailq:
                            emit_tail(*tailq.pop(0))
                if pend is not None:
                    emit_mm3(pend[0], pend[1], pend[2])
                    emit_tail(pend[1], pend[3])

            # --- reduce over the 128 j-offset partitions: out = ACC.T @ ones
            nc.tensor.matmul(warm_ps, ACC, ONES, start=True, stop=True)
            yout = scrp.tile([IPC, 1], f32, tag="yout")
            nc.scalar.copy(yout, warm_ps)
            nc.sync.dma_start(out=yparam[:, :], in_=yout)

    _legalize_waits(nc)
    return nc


_NC_CACHE = {}


def _build_reps(reps):
    if reps not in _NC_CACHE:
        _NC_CACHE[reps] = _build(reps)
    return _NC_CACHE[reps]


def make_in_maps(x, W1, b1, W2, b2, W3, b3):
    x = np.asarray(x, np.float32)
    W1 = np.asarray(W1, np.float32)
    b1 = np.asarray(b1, np.float32)
    W2 = np.asarray(W2, np.float32)
    b2 = np.asarray(b2, np.float32)
    W3 = np.asarray(W3, np.float32)

    hi = x @ W1[:F]                       # [N, H]
    hj = x @ W1[F:]                       # [N, H]
    hib = hi + b1[None, :]                # fold b1
    hj_t = np.ascontiguousarray(hj.T)     # [H, N]
    W3pad = np.zeros((H, OPAD), np.float32)
    W3pad[:, :O] = W3

    cb = np.empty((H, BCOLS), ml_dtypes.bfloat16)
    cb[:, B_HJ:B_HJ + NJ] = hj_t
    cb[:, B_HJ + NJ:B_HJ + NJ + H] = hj_t[:, :H]
    cb[:, B_W2:B_W2 + H] = W2
    cb[:, B_W3:B_W3 + OPAD] = W3pad

    in_maps = []
    for c in range(NCORES):
        hib_c = hib[c * IPC:(c + 1) * IPC].T      # [H, IPC]
        cf = np.empty((H, FCOLS), np.float32)
        cf[:, F_HIB:F_HIB + IPC] = hib_c
        cf[:, F_B2] = b2
        cf[:, F_ONES] = 1.0
        in_maps.append({"cb": cb, "cf": cf})
    return in_maps


def kernel(x, W1, b1, W2, b2, W3, b3):
    b3 = np.asarray(b3, np.float32)
    if np.any(b3 != 0.0):
        # Never hit for this problem (spec fills b3 with zeros); exact
        # numpy fallback keeps the kernel correct for arbitrary inputs.
        return _numpy_ref(
            np.asarray(x, np.float32), np.asarray(W1, np.float32),
            np.asarray(b1, np.float32), np.asarray(W2, np.float32),
            np.asarray(b2, np.float32), np.asarray(W3, np.float32), b3)

    in_maps = make_in_maps(x, W1, b1, W2, b2, W3, b3)
    nc = _build_reps(1)
    res = run_bass_kernel_spmd(nc, in_maps, list(range(NCORES)))
    out = np.concatenate(
        [res.results[c]["y"].reshape(IPC) for c in range(NCORES)]
    ).astype(np.float32)
    return out


def _numpy_ref(x, W1, b1, W2, b2, W3, b3):
    hi = x @ W1[:F]
    hj = x @ W1[F:]
    out = np.empty((N,), np.float32)
    for i in range(N):
        h = np.tanh(hi[i][None, :] + hj + b1[None, :])
        h = np.tanh(h @ W2 + b2[None, :])
        y = np.tanh(h @ W3 + b3[None, :])
        out[i] = y.sum()
    return out


# revision 31
# speedup vs baseline: 1.7552x; 1.7552x over previous
"""Trainium2 Bass kernel for nn_ConvPair (pairwise-MLP message passing).

Reference computation (N=1024 atoms, F=8 feats, H=128 hidden, O=3 out):
    hi = x @ W1[:F];  hj = x @ W1[F:]
    h  = tanh(hi[:,None,:] + hj[None,:,:] + b1)        # [N,N,H]
    h  = tanh(h @ W2 + b2)                             # [N,N,H]
    y  = tanh(h @ W3 + b3)                             # [N,N,O]
    out = y.sum(axis=(1,2))                            # [N]

Sharding: outer atom dim i split across 8 cores (128 i per core); weights and
the (host-precomputed) j-side tables replicated. No cross-core reduction.

Design (vs the fp32 all-ACT baseline at ~790us):
  * all N^2-scale matmuls in bf16 (PE: 1 cycle/row instead of 4 for fp32;
    mm1 moving operand still capped at 512 fp32 PSUM cols per matmul),
  * layer-1 tanh per atom as ONE short ACT instruction using the free
    affine bias port: h1_i = tanh(HJ + hib_i). Short (1K-col) ACT
    instructions measured ~0.35-1.0 ns/col on this part, while LONG ones
    (8K-col batched) run at exactly (N+352)/1.2 ns = 1 col/cycle — so
    batching tanh1 into big instructions is a LOSS; per-atom instructions
    also remove any DVE dependency from the critical path,
  * layer-2 tanh reads PSUM directly with the b2 bias folded in, writes
    bf16; stage-3 uses pairs-on-partitions (h2 128-pair chunk stationary,
    W3pad moving) so tanh3 touches only 32 PSUM cols per atom,
  * per-group (8 atoms) tail: tanh3 in-place on PSUM, DVE reduce, then
    accumulate into ACC (accumulation keeps reps>1 timing variants alive
    against walrus dead-store elimination; reps=1 output is unchanged),
  * final j-offset reduction via one PE matmul (ACC.T @ ones).

Engine budget per core (measured op costs, 128 atoms): PE ~90-140us
(mm1 2x512 rows + 8 stationary swaps for mm3), ACT ~90-290us depending on
short-instruction overlap, DVE ~10us. Rejected alternatives: DVE identity
tanh via reciprocal (nc.vector.reciprocal measured 6.4us per 1K tile);
custom DVE ops (reciprocal_approx_fast) do not compile under this walrus
("ISA wrong length").

b1 is folded into hib host-side; b2 via the ACT bias port; b3 is zeros for
this problem (asserted; a numpy fallback handles the hypothetical nonzero
case exactly).

Wait-discipline: walrus's codegen supports limited semaphore waits per
instruction, so `_legalize_waits` hoists excess waits onto chained NoOps.
"""

import json

import numpy as np
import ml_dtypes
from contextlib import ExitStack

import bass_rust
import concourse.bass as bass
import concourse.tile as tile
from concourse import mybir
from concourse.bass_utils import run_bass_kernel_spmd

f32 = mybir.dt.float32
bf16 = mybir.dt.bfloat16
Tanh = mybir.ActivationFunctionType.Tanh
ADD = mybir.AluOpType.add
MULT = mybir.AluOpType.mult

N, F, H, O = 1024, 8, 128, 3
NCORES = 8
IPC = N // NCORES  # 128 atoms (i) per core
NJ = N             # full j dimension on every core
OPAD = 4           # W3 padded 3 -> 4 cols (pad col = 0)
G = 8              # cohort size (i's per A-group / D-cohort)
NBLK = IPC // (2 * G)  # 8 blocks: each covers one A-group + one D-cohort
NCH = NJ // H      # 8 pair-chunks of 128 j per stage-3 matmul
MMN = 512          # mm1 moving chunk (PSUM bank = 512 fp32)

# bf16 const block column offsets (HJ has a 128-col wrap for mmonly APs)
B_HJ, B_W2, B_W3 = 0, NJ + H, NJ + 2 * H
BCOLS = NJ + 2 * H + OPAD
# f32 const block column offsets
F_HIB, F_B2, F_ONES = 0, IPC, IPC + 1
FCOLS = IPC + 2

# TPB instructions have a single 8-byte events field: 2 sync commands max
# (walrus rejects more).  Queue-engine DMA ops handle their own sync.
_MULTIWAIT_OK = {"DMACopy", "TriggeredCopy", "Call", "ISA"}


def _legalize_waits(nc):
    """Hoist excess semaphore waits from datapath instructions onto chained
    NoOps (one wait each) so every instruction fits walrus's sync budget."""
    j = json.loads(bass_rust.module_to_json_string(nc.m))
    counter = [0]

    def fix_list(insts):
        out = []
        for inst in insts:
            si = inst.get("sync_info")
            waits = (si or {}).get("on_wait", [])
            if si and len(waits) > 1 and inst.get("opcode") not in _MULTIWAIT_OK:
                for w in waits:
                    counter[0] += 1
                    out.append({
                        "debug": inst.get("debug", 0),
                        "engine": inst["engine"],
                        "ins": [],
                        "outs": [],
                        "name": f"W-hoist-{counter[0]}",
                        "opcode": "NoOp",
                        "sync_info": {"on_update": [], "on_wait": [w]},
                    })
                si["on_wait"] = []
            out.append(inst)
        return out

    def walk(o):
        if isinstance(o, dict):
            if "instructions" in o and isinstance(o["instructions"], list):
                o["instructions"] = fix_list(o["instructions"])
            for v in o.values():
                walk(v)
        elif isinstance(o, list):
            for v in o:
                walk(v)

    walk(j)
    nc.m = bass_rust.module_from_json_string(json.dumps(j))
    return counter[0]


def _build(reps=1):
    """Build the per-core Bass program (SPMD: same program, per-core data).

    reps > 1 repeats the main loop (recomputing identical results); used
    only for differential timing.

    NN_CONVPAIR_TIMING_VARIANT env (timing experiments ONLY — wrong results): "batched" uses DVE
    adds + one 8K ACT tanh per group; "not1"/"not2"/"nomm3" skip stages;
    "t1only"/"t1only_batched" measure the tanh1 stream alone; "mmonly"
    measures the PE stream alone."""
    import os
    variant = os.environ.get("NN_CONVPAIR_TIMING_VARIANT", "full")
    nc = bass.Bass()
    cbparam = nc.declare_dram_parameter("cb", [H, BCOLS], bf16, isOutput=False)
    cfparam = nc.declare_dram_parameter("cf", [H, FCOLS], f32, isOutput=False)
    yparam = nc.declare_dram_parameter("y", [IPC, 1], f32, isOutput=True)

    with tile.TileContext(nc) as tc:
        with ExitStack() as ctx:
            cbp = ctx.enter_context(tc.tile_pool(name="cbp", bufs=1))
            cfp = ctx.enter_context(tc.tile_pool(name="cfp", bufs=1))
            h1p = ctx.enter_context(tc.tile_pool(name="h1p", bufs=3))
            h2p = ctx.enter_context(tc.tile_pool(name="h2p", bufs=3))
            accp = ctx.enter_context(tc.tile_pool(name="accp", bufs=1))
            scrp = ctx.enter_context(tc.tile_pool(name="scrp", bufs=1))
            # PSUM budget (8 banks): ps1 2 bufs x 2 banks + ps3 3 x 1
            # + warm/fin 1
            ps1p = ctx.enter_context(tc.tile_pool(name="ps1p", bufs=2,
                                                  space="PSUM"))
            ps3p = ctx.enter_context(tc.tile_pool(name="ps3p", bufs=3,
                                                  space="PSUM"))
            psW = ctx.enter_context(tc.tile_pool(name="psW", bufs=1,
                                                 space="PSUM"))

            CB = cbp.tile([H, BCOLS], bf16)
            nc.sync.dma_start(out=CB, in_=cbparam[:, :])
            CF = cfp.tile([H, FCOLS], f32)
            nc.sync.dma_start(out=CF, in_=cfparam[:, :])

            HJ = CB[:, B_HJ:B_HJ + NJ]
            W2 = CB[:, B_W2:B_W2 + H]
            W3 = CB[:, B_W3:B_W3 + OPAD]
            B2 = CF[:, F_B2:F_B2 + 1]
            ONES = CF[:, F_ONES:F_ONES + 1]

            ACC = accp.tile([H, IPC], f32)          # [j-offset, i] partials
            warm = scrp.tile([H, 1], f32, tag="warm")
            warmb = scrp.tile([H, 1], bf16, tag="warmb")

            # --- warmups: every engine observes both const DMAs on
            # single-wait instructions; first Tanh loads the ACT table.
            nc.scalar.activation(out=warm, in_=B2, func=Tanh)
            nc.scalar.activation(out=warmb, in_=CB[:, 0:1], func=Tanh)
            warm_ps = psW.tile([IPC, 1], f32, tag="wf")
            nc.tensor.matmul(warm_ps[0:1, 0:1], CB[:, B_W2:B_W2 + 1],
                             CB[:, B_W2:B_W2 + 1], start=True, stop=True)
            nc.tensor.matmul(warm_ps[0:1, 0:1], ONES, ONES,
                             start=True, stop=True)
            warmd = scrp.tile([H, 1], f32, tag="warmd")
            nc.vector.tensor_scalar_add(warmd, CB[:, 0:1], CF[:, 0:1])
            # ACC is accumulated into (not overwritten) so that with reps>1
            # every rep stays live (defeats walrus DCE in timing variants).
            nc.vector.memset(ACC, 0.0)
            redp = ctx.enter_context(tc.tile_pool(name="redp", bufs=2))

            def emit_group_h1(g):
                """Group h1 production. Default: per-atom ACT tanh via the
                bias port, split into 512-col halves (short ACT instructions
                overlap on HW; long ones run at 1 col/cycle).
                variant=batched: DVE adds + one big ACT tanh (slow: 8K ACT)."""
                h1g = h1p.tile([H, G, NJ], bf16)
                for k in range(G):
                    t = G * g + k
                    bias = CF[:, F_HIB + t:F_HIB + t + 1]
                    if variant in ("batched", "t1only_batched"):
                        nc.vector.tensor_scalar_add(h1g[:, k, :], HJ, bias)
                    else:
                        nc.scalar.activation(out=h1g[:, k, :], in_=HJ,
                                             func=Tanh, bias=bias)
                if variant in ("batched", "t1only_batched"):
                    nc.scalar.activation(out=h1g[:, :, :], in_=h1g[:, :, :],
                                         func=Tanh)
                return h1g

            mm1_pm = (mybir.MatmulPerfMode.DoublePixel
                      if variant == "dpix" else None)

            def emit_mm1_t2(h1):
                """PE mm1 -> ACT tanh2 for one atom; mm3 deferred so the
                PE can fill tanh2's latency with the PREVIOUS atom's mm3s
                (one-atom software pipeline)."""
                ps1 = ps1p.tile([H, NJ], f32)
                h2 = h2p.tile([H, NJ], bf16)
                for m in range(NJ // MMN):
                    sl = slice(m * MMN, (m + 1) * MMN)
                    nc.tensor.matmul(ps1[:, sl], W2, h1[:, sl],
                                     start=True, stop=True, perf_mode=mm1_pm)
                if variant != "not2":
                    nc.scalar.activation(out=h2, in_=ps1, func=Tanh, bias=B2)
                return h2

            def emit_mm3(h2, ps3, slot):
                for c in range(NCH):
                    nc.tensor.matmul(ps3[:, slot, c, :],
                                     h2[:, c * H:(c + 1) * H], W3,
                                     start=True, stop=True)

            def emit_mm(h1, ps3, slot):
                emit_mm3(emit_mm1_t2(h1), ps3, slot)

            def emit_tail(ps3, col0):
                """ACT tanh3 in-place on PSUM; DVE reduce + accumulate."""
                nc.scalar.activation(out=ps3[:, :, :, :], in_=ps3[:, :, :, :],
                                     func=Tanh)
                red = redp.tile([H, G], f32)
                nc.vector.tensor_reduce(
                    out=red,
                    in_=ps3.rearrange("p g c o -> p g (c o)"),
                    axis=mybir.AxisListType.X, op=ADD)
                nc.vector.tensor_tensor(out=ACC[:, col0:col0 + G],
                                        in0=ACC[:, col0:col0 + G],
                                        in1=red, op=ADD)

            for _ in range(reps):
                if variant == "null":
                    break
                if variant in ("t1only", "t1only_batched"):
                    # measure the tanh1 stream alone; accumulating reduce
                    # keeps every h1g live (defeats DCE).
                    for g in range(IPC // G):
                        h1g = emit_group_h1(g)
                        red = redp.tile([H, G], f32)
                        # touch every k-slice so no tanh1 is dead code
                        nc.vector.tensor_reduce(
                            out=red, in_=h1g[:, :, 0:2],
                            axis=mybir.AxisListType.X, op=ADD)
                        nc.vector.tensor_tensor(out=ACC[:, g * G:(g + 1) * G],
                                                in0=ACC[:, g * G:(g + 1) * G],
                                                in1=red, op=ADD)
                    continue
                if variant == "mmonly":
                    # measure the PE stream alone; unique AP offsets per i
                    # (HJ wrap region) defeat elision; no tanh1/tanh2.
                    for g in range(IPC // G):
                        ps3 = ps3p.tile([H, G, NCH, OPAD], f32, tag="s3")
                        for k in range(G):
                            t = G * g + k
                            off = t % H
                            ps1 = ps1p.tile([H, NJ], f32)
                            for m in range(NJ // MMN):
                                nc.tensor.matmul(
                                    ps1[:, m * MMN:(m + 1) * MMN], W2,
                                    CB[:, off + m * MMN:off + (m + 1) * MMN],
                                    start=True, stop=True)
                            for c in range(NCH):
                                nc.tensor.matmul(
                                    ps3[:, k, c, :],
                                    CB[:, off + c * H:off + (c + 1) * H], W3,
                                    start=True, stop=True)
                        emit_tail(ps3, G * g)
                    continue
                # software pipeline: group g+1's h1 (DVE adds + one batched
                # ACT tanh) is produced while group g runs mm1/tanh2/mm3.
                h1g_next = None if variant == "not1" else emit_group_h1(0)
                for g in range(IPC // G):
                    h1g = h1g_next
                    if g + 1 < IPC // G and variant != "not1":
                        h1g_next = emit_group_h1(g + 1)
                    ps3 = ps3p.tile([H, G, NCH, OPAD], f32, tag="s3")
                    for k in range(G):
                        h1 = HJ if variant == "not1" else h1g[:, k, :]
                        if variant == "nomm3":
                            emit_mm1_t2(h1)
                            continue
                        emit_mm(h1, ps3, k)
                    if variant != "nomm3":
                        emit_tail(ps3, G * g)

            # --- reduce over the 128 j-offset partitions: out = ACC.T @ ones
            nc.tensor.matmul(warm_ps, ACC, ONES, start=True, stop=True)
            yout = scrp.tile([IPC, 1], f32, tag="yout")
            nc.scalar.copy(yout, warm_ps)
            nc.sync.dma_start(out=yparam[:, :], in_=yout)

    _legalize_waits(nc)
    return nc


_NC_CACHE = {}


def _build_reps(reps):
    if reps not in _NC_CACHE:
        _NC_CACHE[reps] = _build(reps)
    return _NC_CACHE[reps]


def make_in_maps(x, W1, b1, W2, b2, W3, b3):
    x = np.asarray(x, np.float32)
    W1 = np.asarray(W1, np.float32)
    b1 = np.asarray(b1, np.float32)
    W2 = np.asarray(W2, np.float32)
    b2 = np.asarray(b2, np.float32)
    W3 = np.asarray(W3, np.float32)

    hi = x @ W1[:F]                       # [N, H]
    hj = x @ W1[F:]                       # [N, H]
    hib = hi + b1[None, :]                # fold b1
    hj_t = np.ascontiguousarray(hj.T)     # [H, N]
    W3pad = np.zeros((H, OPAD), np.float32)
    W3pad[:, :O] = W3

    cb = np.empty((H, BCOLS), ml_dtypes.bfloat16)
    cb[:, B_HJ:B_HJ + NJ] = hj_t
    cb[:, B_HJ + NJ:B_HJ + NJ + H] = hj_t[:, :H]
    cb[:, B_W2:B_W2 + H] = W2
    cb[:, B_W3:B_W3 + OPAD] = W3pad

    in_maps = []
    for c in range(NCORES):
        hib_c = hib[c * IPC:(c + 1) * IPC].T      # [H, IPC]
        cf = np.empty((H, FCOLS), np.float32)
        cf[:, F_HIB:F_HIB + IPC] = hib_c
        cf[:, F_B2] = b2
        cf[:, F_ONES] = 1.0
        in_maps.append({"cb": cb, "cf": cf})
    return in_maps


def kernel(x, W1, b1, W2, b2, W3, b3):
    b3 = np.asarray(b3, np.float32)
    if np.any(b3 != 0.0):
        # Never hit for this problem (spec fills b3 with zeros); exact
        # numpy fallback keeps the kernel correct for arbitrary inputs.
        return _numpy_ref(
            np.asarray(x, np.float32), np.asarray(W1, np.float32),
            np.asarray(b1, np.float32), np.asarray(W2, np.float32),
            np.asarray(b2, np.float32), np.asarray(W3, np.float32), b3)

    in_maps = make_in_maps(x, W1, b1, W2, b2, W3, b3)
    nc = _build_reps(1)
    res = run_bass_kernel_spmd(nc, in_maps, list(range(NCORES)))
    out = np.concatenate(
        [res.results[c]["y"].reshape(IPC) for c in range(NCORES)]
    ).astype(np.float32)
    return out


def _numpy_ref(x, W1, b1, W2, b2, W3, b3):
    hi = x @ W1[:F]
    hj = x @ W1[F:]
    out = np.empty((N,), np.float32)
    for i in range(N):
        h = np.tanh(hi[i][None, :] + hj + b1[None, :])
        h = np.tanh(h @ W2 + b2[None, :])
        y = np.tanh(h @ W3 + b3[None, :])
        out[i] = y.sum()
    return out
